# revision 41
# baseline (speedup 1.0000x reference)
"""Trainium2 Bass kernel for a transformer decoder layer (pre-norm, eval mode).

Computation (per batch row):
    x = x + MHA(LN1(x), LN1(x), LN1(x), mask)      # masked self-attention
    x = x + MHA(LN2(x), enc, enc, None)            # cross-attention
    x = x + W2 @ relu(W1 @ LN3(x) + b1) + b2       # FFN

Shapes: B=4, S=2048, D=512, H=8 heads (dk=64), FF=1024, fp32.

Sharding: 8 cores = (batch b, query-half). Each core computes 1024 query rows
of one batch, with the full 2048-token K/V context. No collectives needed.

v5 design (HW-measured: PE streams run at ~cost-model speed, elementwise
streams ~1.5-1.9x slower -> spread exp/mask work over Act+DVE+GpSimd and
keep elementwise op count low at [P,1024] granularity):
  - Self-attn (bf16 weights+V: V quantization error does NOT average out
    when the diagonal q.q weight dominates a softmax row — host emulation
    showed e4m3 V alone costs 2.6e-2). Per (key-tile, head) unit, engine
    pattern SELF_PAT picks: 'g' = Act exact exp + GpSimd 0/1-mask multiply
    (SBUF-only, runs beside DVE), 'd' = mask folded into scores via a PE
    identity matmul (sc = I.T @ maskbias + K.T @ Q, maskbias=-30000) then
    a single DVE SCH16 Schraudolph op (u16 = s*A+B == bf16 bits of
    exp(s/8); saturating u16 convert maps masked scores to +0.0 on HW).
  - Cross-attn: fp8 DoubleRow attn@V (2 key tiles/matmul, V=e4m3+ones
    column for the denominator). exp halves split Act/DVE per
    CROSS_ACT_NUM: even heads e4m3 (Act exp/16 or u8 e4m3-Schraudolph),
    odd heads e5m2 (Act exp/1024 or u8 e5m2-Schraudolph).
  - Both attentions stage un-normalized accumulators (denominator in PSUM
    row 64) -> Act copy to oT -> PE 65-row transpose -> DVE fused
    divide-add into the residual stream.
  - QK matmuls of a head pair sit at partition offsets 0/64 -> auto row-
    group packing. Scores PSUM 2x [128,1024] + accumulators 2x [65,1024].
  - x1T/x2T/x3T transposed layouts via DMA-engine XBAR transposes (idle
    capacity) instead of PE transpose + PSUM copy.
  - LN rstd = exp(-0.5*ln(var+eps)) + an act-table preference patch keeps
    the Act table on natural_log_exp_and_others the whole kernel (the
    default greedy set picker thrashed 45 x ~2.7us table loads).

Measured (repeat-slope, 8 cores): 574,893 ns; rel err 4.39e-3 (gate 2e-2).
Baseline at session start: 800,202 ns.
"""

import functools

import numpy as np

B = 4
S = 2048
D = 512
H = 8
DK = 64
DFF = 1024
Q = 1024  # queries per core
P = 128
TS = S // P  # 16 key tiles
TQ = Q // P  # 8 query tiles
EPS = 1e-5
N_CORES = 8

# Schraudolph exp bit tricks (round-to-nearest + saturating converts on HW):
# cross j=1: u8 = s*A + B ~ e5m2 bits of exp(s/8)/1024 (B = 60.25 - 4*10;
# /1024 headroom for any score); self: u16 = s*A16 + B16 ~ bf16 bits of
# exp(s/8) (8-bit mantissa — self V error doesn't average out over keys, so
# self attn stays bf16; see host-emulation: V e4m3 alone costs 2.6e-2).
SCH_A = 0.7213475
SCH_B_E5 = 20.25
SCH16_A = 23.0831207
SCH16_B = 16251.0
NEG_LN1024 = -6.9314718  # Act e5m2 path: exp(0.125*s - ln1024)
MASK_BIAS = -30000.0

# Per-unit engine pattern for self-attn (cycled): a=Act exp + DVE mask,
# g=Act exp + GpSimd mask, d=PE mask + DVE SCH16
SELF_PAT = "gd"
# cross: e4m3 Schraudolph (u8 = s*A4 + B4 ~ e4m3 bits of exp(s/8)/16) lets
# BOTH head types split Act/DVE: exp half on Act if (idx % DEN) < NUM
SCH4_A = 1.44269504
SCH4_B = 24.0
CROSS_ACT_NUM, CROSS_ACT_DEN = 9, 16


@functools.lru_cache(maxsize=None)
def _build_graph(a1, a2, a3, use_b1, use_b2, repeat=1, no_dma=False,
                 phases="1,2,t2,3,t3,4"):
    """Build the (SPMD, per-core) Bass graph. aN: apply lnN gamma/beta.

    phases: comma-set of {1,2,t2,3,t3,4} — ablation knob for HW phase
    attribution (timing only; output is garbage unless all are on).
    """
    PH = set(phases.split(","))
    from contextlib import ExitStack

    import concourse.bacc as bacc
    import concourse.mybir as mybir
    import concourse.tile as tile
    from concourse.masks import make_identity

    f32 = mybir.dt.float32
    bf16 = mybir.dt.bfloat16
    f8e4 = mybir.dt.float8e4
    f8e5 = mybir.dt.float8e5
    u8 = mybir.dt.uint8
    u16 = mybir.dt.uint16
    AF = mybir.ActivationFunctionType
    OP = mybir.AluOpType
    DR = mybir.MatmulPerfMode.DoubleRow

    nc = bacc.Bacc("TRN2", target_bir_lowering=False, debug=False)

    x_d = nc.dram_tensor("x", [S, D], f32, kind="ExternalInput")
    xq_d = nc.dram_tensor("xq", [P, TQ, D], f32, kind="ExternalInput")
    encT_d = nc.dram_tensor("encT", [P, 4, S], bf16, kind="ExternalInput")
    # head stride padded to 80 (16B-aligned for DoubleRow ldweights);
    # col 64 = ones (softmax denominator), cols 65..79 = zero
    encv_d = nc.dram_tensor("encv", [P, TS, H, 80], f8e4, kind="ExternalInput")
    # additive mask bias, transposed: 0 (unmasked) / -30000 (masked)
    maskT_d = nc.dram_tensor("maskT", [P, TS, Q], bf16, kind="ExternalInput")
    # multiplicative 0/1 mask (for self Act-halves: exp then DVE multiply)
    maskM_d = nc.dram_tensor("maskM", [P, TS, Q], bf16, kind="ExternalInput")
    w1_d = nc.dram_tensor("w1", [P, 4, DFF], bf16, kind="ExternalInput")
    w2_d = nc.dram_tensor("w2", [P, DFF // P, D], bf16, kind="ExternalInput")
    ln_d = {}
    for i, a in ((1, a1), (2, a2), (3, a3)):
        if a:
            ln_d[i] = (
                nc.dram_tensor(f"ln{i}gr", [P, D], f32, kind="ExternalInput"),
                nc.dram_tensor(f"ln{i}br", [P, D], f32, kind="ExternalInput"),
            )
    if use_b1:
        b1t_d = nc.dram_tensor("b1t", [P, DFF // P], f32, kind="ExternalInput")
    if use_b2:
        b2r_d = nc.dram_tensor("b2r", [P, D], f32, kind="ExternalInput")
    out_d = nc.dram_tensor("out", [P, TQ, D], f32, kind="ExternalOutput")

    with tile.TileContext(nc) as tc, ExitStack() as ctx:
        const = ctx.enter_context(tc.tile_pool(name="const", bufs=1))
        big = ctx.enter_context(tc.tile_pool(name="big", bufs=1))
        work = ctx.enter_context(tc.tile_pool(name="work", bufs=4))
        work3 = ctx.enter_context(tc.tile_pool(name="work3", bufs=4))
        atp = ctx.enter_context(tc.tile_pool(name="atp", bufs=3))
        pssc = ctx.enter_context(tc.tile_pool(name="pssc", bufs=2, space="PSUM"))
        psacc = ctx.enter_context(tc.tile_pool(name="psacc", bufs=2, space="PSUM"))

        identb_p = const.tile([P, P], bf16)
        make_identity(nc, identb_p)
        epst = const.tile([P, 1], f32)
        nc.vector.memset(epst, EPS)
        nege = const.tile([P, 1], f32)
        nc.vector.memset(nege, -2.7725887)  # -ln(16): exp/16 for e4m3 range
        negl256 = const.tile([P, 1], f32)
        nc.vector.memset(negl256, NEG_LN1024)
        ln_sb = {}
        for i, (gd, bd) in ln_d.items():
            g = const.tile([P, D], f32, tag=f"ln{i}g")
            b = const.tile([P, D], f32, tag=f"ln{i}b")
            nc.sync.dma_start(g, gd.ap())
            nc.sync.dma_start(b, bd.ap())
            ln_sb[i] = (g, b)
        if use_b1:
            b1t = const.tile([P, DFF // P], f32)
            nc.sync.dma_start(b1t, b1t_d.ap())
        if use_b2:
            b2r = const.tile([P, D], f32)
            nc.sync.dma_start(b2r, b2r_d.ap())

        def _emit_iteration():
            # persistent / phase-aliased tensors (same tag = same SBUF slot)
            x1T = big.tile([P, 4, S], bf16, tag="A")            # LN1(x)^T
            v_self = big.tile([P, TS, H, DK + 1], bf16, tag="B")
            maskM = big.tile([P, TS, Q], bf16, tag="MM")
            maskT = big.tile([P, TS, Q], bf16, tag="C")
            xbuf = big.tile([P, TQ, D], f32, tag="X")          # evolving query rows

            encT = big.tile([P, 4, S], bf16, tag="ENCT")
            encv = big.tile([P, TS, H, 80], f8e4, tag="ENCV")
            # token rows 1024..2047 stage in the slot oT takes over later
            # (first oT write happens after their last read in phase 1)
            x_c2 = big.tile([P, 2, 4, D], f32, tag="OT", name="x_c2")
            x_cs = [x_c2[:, 0], x_c2[:, 1]]
            if no_dma:
                nc.gpsimd.memset(xbuf[:], 0.01)
                for c in range(2):
                    nc.gpsimd.memset(x_cs[c][:], 0.01)
                nc.gpsimd.memset(maskT[:], 0.0)
                nc.gpsimd.memset(maskM[:], 1.0)
                nc.gpsimd.memset(encT[:], 0.01)
                nc.gpsimd.memset(encv[:].bitcast(u8), 52)
            else:
                nc.sync.dma_start(xbuf[:, 0:4, :], xq_d.ap()[:, 0:4, :])
                nc.sync.dma_start(xbuf[:, 4:8, :], xq_d.ap()[:, 4:8, :])
                for c in range(2):
                    nc.sync.dma_start(
                        x_cs[c], x_d.ap()[(c + 2) * 4 * P : (c + 3) * 4 * P].rearrange(
                            "(t p) d -> p t d", p=P
                        )
                    )
                nc.sync.dma_start(maskT, maskT_d.ap())
                nc.sync.dma_start(maskM, maskM_d.ap())
                nc.sync.dma_start(encT, encT_d.ap())
                nc.sync.dma_start(encv, encv_d.ap())
            # ones column for the softmax denominator row
            nc.vector.memset(v_self[:, :, :, DK : DK + 1], 1.0)

            def layer_norm_tile(x_t, which, use_act=False):
                """LN of a [P, D] tile (tokens on partitions) -> bf16 tile."""
                xn = work3.tile([P, D], bf16, tag="xn")
                if not use_act:
                    stats = work.tile([P, 6], f32, tag="stats")
                    nc.vector.bn_stats(stats, x_t)
                    mv = work.tile([P, 2], f32, tag="mv")
                    nc.vector.bn_aggr(mv, stats)
                    mu = mv[:, 0:1]
                    var = mv[:, 1:2]
                else:
                    tmp = work3.tile([P, D], f32, tag="xtmp", bufs=2)
                    sums = work.tile([P, 1], f32, tag="sums")
                    nc.scalar.activation(tmp, x_t, AF.Copy, accum_out=sums)
                    sumsq = work.tile([P, 1], f32, tag="sumsq")
                    nc.scalar.activation(tmp, x_t, AF.Square, accum_out=sumsq)
                    mu = work.tile([P, 1], f32, tag="mu")
                    nc.vector.tensor_scalar_mul(mu, sums, 1.0 / D)
                    musq = work.tile([P, 1], f32, tag="musq")
                    nc.vector.tensor_mul(musq, mu, mu)
                    var = work.tile([P, 1], f32, tag="var")
                    nc.vector.tensor_scalar(
                        var, sumsq, scalar1=1.0 / D, scalar2=musq,
                        op0=OP.mult, op1=OP.subtract,
                    )
                # rstd = exp(-0.5*ln(var+eps)) — stays on the exp/ln Act
                # table set (sqrt would force a ~2.7us table swap)
                lnv = work.tile([P, 1], f32, tag="lnv")
                nc.scalar.activation(lnv, var, AF.Ln, bias=epst[:])
                rstd = work.tile([P, 1], f32, tag="rstd")
                nc.scalar.activation(rstd, lnv, AF.Exp, scale=-0.5)
                if which in ln_sb:
                    xf = work3.tile([P, D], f32, tag="xnf")
                    nc.vector.tensor_scalar(
                        xf, x_t, scalar1=mu, scalar2=rstd,
                        op0=OP.subtract, op1=OP.mult,
                    )
                    g, b = ln_sb[which]
                    nc.vector.tensor_mul(xf, xf, g)
                    nc.vector.tensor_add(xf, xf, b)
                    nc.vector.tensor_copy(xn, xf)
                else:
                    nc.vector.tensor_scalar(
                        xn, x_t, scalar1=mu, scalar2=rstd,
                        op0=OP.subtract, op1=OP.mult,
                    )
                return xn

            # ---- phase 1: LN1 over all 16 token tiles; build x1T and V_self
            for t in (range(TS) if "1" in PH else ()):
                if t < 8:
                    x_t = xbuf[:, t, :]
                else:
                    x_t = x_cs[(t - 8) // 4][:, (t - 8) % 4, :]
                x1_t = layer_norm_tile(x_t, 1, use_act=(t % 2 == 1))
                vdst = v_self[:, t, :, 0:DK]
                vsrc = x1_t[:].rearrange("p (h d) -> p h d", h=H)
                if t % 2 == 0:
                    nc.vector.tensor_copy(vdst, vsrc)
                else:
                    nc.scalar.copy(vdst, vsrc)
                # DMA XBAR transpose straight into the [d, tok] layout
                nc.sync.dma_start_transpose(
                    x1T[:, :, t * P : (t + 1) * P], x1_t[:]
                )

            # takes over the x_c2 slot (last read above); row 64 carries the
            # softmax denominator (normalize folded into the merge)
            oT = big.tile([DK + 1, H, Q], bf16, tag="OT", name="oT")

            identb65 = identb_p[0 : DK + 1, 0 : DK + 1]

            def merge_pair(h0):
                """Un-normalized staged heads: transpose (with denom row),
                then scale by 1/denom inside the residual add."""
                for qt in range(TQ):
                    pso = pssc.tile(
                        [P, 2, DK + 2], bf16, tag="sc", name=f"pso{h0}_{qt}"
                    )
                    for j in range(2):
                        nc.tensor.transpose(
                            pso[:, j, 0 : DK + 1],
                            oT[:, h0 + j, qt * P : (qt + 1) * P],
                            identb65,
                        )
                    rc = work.tile([P, 2, 1], f32, tag="rc")
                    nc.vector.reciprocal(rc[:], pso[:, :, DK : DK + 1])
                    for j in range(2):
                        nc.vector.scalar_tensor_tensor(
                            xbuf[:, qt, (h0 + j) * DK : (h0 + j + 1) * DK],
                            pso[:, j, 0:DK],
                            rc[:, j, :],
                            xbuf[:, qt, (h0 + j) * DK : (h0 + j + 1) * DK],
                            op0=OP.mult, op1=OP.add,
                        )

            def ln_tail(which, box, dtype, compute=True):
                """Post-attention LN of all query tiles -> transposed box."""
                if not box:
                    box.append(
                        big.tile([P, 4, Q], dtype, tag="A" if which == 2 else "B",
                                 name=f"x{which}T")
                    )
                if not compute:
                    return
                for qt in range(TQ):
                    xn = layer_norm_tile(xbuf[:, qt], which)
                    nc.sync.dma_start_transpose(
                        box[0][:, :, qt * P : (qt + 1) * P], xn[:]
                    )

            def attn(qT, kT, vT, use_mb, e4_head, act_num, act_den, label):
                """Unified attention: per (head-pair, 2-key-tile group):
                [mask matmuls] -> QK (row-packed) -> exp halves -> fp8 DR AV.

                e4_head(h): True -> all-Act e4m3 exp (1/16); False -> e5m2
                (1/256), halves split Act/DVE by the (act_num, act_den) ratio.
                """
                cnt = [0]
                for h0 in range(0, H, 2):
                    accs = [
                        psacc.tile([DK + 1, Q], f32, tag="acc",
                                   name=f"{label}acc{h0}"),
                        psacc.tile([DK + 1, Q], f32, tag="acc",
                                   name=f"{label}acc{h0 + 1}"),
                    ]
                    for i in range(TS // 2):
                        for j in range(2):
                            h = h0 + j
                            hp = (h % 2) * DK
                            hf = h // 2
                            e4 = e4_head(h)
                            at2 = atp.tile(
                                [P, 2, Q], f8e4 if e4 else u8,
                                tag=f"at2_{j}", name=f"{label}at2_{h0}_{j}_{i}",
                            )
                            at2v = at2 if e4 else at2.bitcast(f8e5)
                            for t in range(2):
                                s = 2 * i + t
                                kTr = kT[hp : hp + DK, hf, s * P : (s + 1) * P]
                                # one 2-bank tile, matmul+exp per half: 2
                                # tiles in flight == 4 halves pipelined
                                sc = pssc.tile(
                                    [P, Q], f32, tag="sc",
                                    name=f"{label}sc{j}_{t}",
                                )
                                for n2 in range(2):
                                    sl = slice(n2 * 512, (n2 + 1) * 512)
                                    nc.tensor.matmul(
                                        sc[:, sl], lhsT=kTr,
                                        rhs=qT[hp : hp + DK, hf, sl],
                                        start=True, stop=True,
                                    )
                                    on_act = cnt[0] % act_den < act_num
                                    cnt[0] += 1
                                    if on_act:
                                        nc.scalar.activation(
                                            at2v[:, t, sl], sc[:, sl], AF.Exp,
                                            scale=0.125,
                                            bias=nege[:] if e4 else negl256[:],
                                        )
                                    else:
                                        nc.vector.tensor_scalar(
                                            at2.bitcast(u8)[:, t, sl],
                                            sc[:, sl],
                                            scalar1=SCH4_A if e4 else SCH_A,
                                            scalar2=SCH4_B if e4 else SCH_B_E5,
                                            op0=OP.mult, op1=OP.add,
                                        )
                            rhs_f8 = at2 if e4 else at2.bitcast(f8e5)
                            for n2 in range(2):
                                nc.tensor.matmul(
                                    accs[j][:, n2 * 512 : (n2 + 1) * 512],
                                    lhsT=vT[:, 2 * i : 2 * i + 2, h, 0 : DK + 1],
                                    rhs=rhs_f8[:, :, n2 * 512 : (n2 + 1) * 512],
                                    start=(i == 0), stop=(i == TS // 2 - 1),
                                    perf_mode=DR,
                                )
                    for j in range(2):
                        nc.scalar.copy(oT[:, h0 + j, :], accs[j][:])
                    merge_pair(h0)

            def attn_self():
                """bf16 self-attention (V quantization error doesn't average
                out when the diagonal weight dominates -> no fp8 here).
                One [P,1024] elementwise op per engine per unit, engine
                picked per unit from SELF_PAT:
                  'a': Act exact exp -> bf16, DVE 0/1-mask multiply
                  'g': Act exact exp -> bf16, GpSimd 0/1-mask multiply
                       (SBUF-only op; runs beside DVE's dedicated ports)
                  'd': PE adds mask bias into PSUM, DVE SCH16 Schraudolph
                       (u16 = s*A+B == bf16 bits of exp(s/8); masked args
                       saturate to +0.0 on HW)
                """
                cnt = [0]
                for h0 in range(0, H, 2):
                    accs = [
                        psacc.tile([DK + 1, Q], f32, tag="acc",
                                   name=f"sacc{h0}"),
                        psacc.tile([DK + 1, Q], f32, tag="acc",
                                   name=f"sacc{h0 + 1}"),
                    ]
                    for s in range(TS):
                        # emit masks first, then the j0/j1 QK matmuls
                        # back-to-back per half: the two heads sit at
                        # partition offsets 0/64 -> the PE runs each QK
                        # pair concurrently (row-group packing)
                        for j in range(2):
                            h = h0 + j
                            hp = (h % 2) * DK
                            hf = h // 2
                            eng = SELF_PAT[cnt[0] % len(SELF_PAT)]
                            cnt[0] += 1
                            at = atp.tile([P, Q], bf16, tag=f"at_{j}",
                                          name=f"sat_{h0}_{j}_{s}", bufs=4)
                            kTr = x1T[hp : hp + DK, hf, s * P : (s + 1) * P]
                            sc = pssc.tile([P, Q], f32, tag="sc",
                                           name=f"ssc{j}")
                            pe_mask = eng in ("d", "p")
                            for n2 in range(2):
                                sl = slice(n2 * 512, (n2 + 1) * 512)
                                if pe_mask:
                                    nc.tensor.matmul(
                                        sc[:, sl], lhsT=identb_p[:],
                                        rhs=maskT[:, s, sl],
                                        start=True, stop=False,
                                    )
                                nc.tensor.matmul(
                                    sc[:, sl], lhsT=kTr,
                                    rhs=x1T[hp : hp + DK, hf, sl],
                                    start=not pe_mask, stop=True,
                                )
                            if eng == "d":
                                nc.vector.tensor_scalar(
                                    at.bitcast(u16)[:], sc[:],
                                    scalar1=SCH16_A, scalar2=SCH16_B,
                                    op0=OP.mult, op1=OP.add,
                                )
                            elif eng == "p":
                                # PE-masked scores underflow to 0 in exp:
                                # no mask op at all for this unit
                                nc.scalar.activation(
                                    at[:], sc[:], AF.Exp, scale=0.125,
                                )
                            else:
                                nc.scalar.activation(
                                    at[:], sc[:], AF.Exp, scale=0.125,
                                )
                                mul = (nc.gpsimd if eng == "g"
                                       else nc.vector).tensor_mul
                                mul(at[:], at[:], maskM[:, s, :])
                            for n2 in range(2):
                                sl = slice(n2 * 512, (n2 + 1) * 512)
                                nc.tensor.matmul(
                                    accs[j][:, sl],
                                    lhsT=v_self[:, s, h, :],
                                    rhs=at[:, sl],
                                    start=(s == 0), stop=(s == TS - 1),
                                )
                    for j in range(2):
                        nc.scalar.copy(oT[:, h0 + j, :], accs[j][:])
                    merge_pair(h0)

            # ---- phase 2: masked self-attention (bf16)
            if "2" in PH:
                attn_self()
            x2T_box = []
            ln_tail(2, x2T_box, bf16, compute="t2" in PH)

            # ---- phase 3: cross-attention (j=0 heads e4m3 on Act)
            if "3" in PH:
                attn(x2T_box[0], encT, encv, False, lambda h: h % 2 == 0,
                     CROSS_ACT_NUM, CROSS_ACT_DEN, "c")
            x3T_box = []
            ln_tail(3, x3T_box, bf16, compute="t3" in PH)

            if "4" not in PH:
                return
            # ---- phase 4: FFN (bf16: fp8 here dominated the error budget)
            x3T = x3T_box[0]
            w1sb = big.tile([P, 4, DFF], bf16, tag="A")
            w2sb = big.tile([P, DFF // P, D], bf16, tag="OT")
            if no_dma:
                nc.gpsimd.memset(w1sb[:], 0.01)
                nc.gpsimd.memset(w2sb[:], 0.01)
            else:
                nc.sync.dma_start(w1sb, w1_d.ap())
                nc.sync.dma_start(w2sb, w2_d.ap())
            hT = big.tile([P, DFF // P, Q], bf16, tag="C")
            for f in range(DFF // P):
                for n2 in range(2):
                    hps = pssc.tile([P, 512], f32, tag="sc", name=f"hps{n2}")
                    for ft in range(4):
                        nc.tensor.matmul(
                            hps[:],
                            lhsT=w1sb[:, ft, f * P : (f + 1) * P],
                            rhs=x3T[:, ft, n2 * 512 : (n2 + 1) * 512],
                            start=(ft == 0), stop=(ft == 3),
                        )
                    bias = b1t[:, f : f + 1] if use_b1 else 0.0
                    nc.scalar.activation(
                        hT[:, f, n2 * 512 : (n2 + 1) * 512], hps[:],
                        AF.Relu, bias=bias,
                    )
            for qt in range(TQ):
                ops = pssc.tile([P, D], f32, tag="sc")
                for f in range(DFF // P):
                    nc.tensor.matmul(
                        ops[:],
                        lhsT=hT[:, f, qt * P : (qt + 1) * P],
                        rhs=w2sb[:, f, :],
                        start=(f == 0), stop=(f == DFF // P - 1),
                    )
                nc.vector.tensor_add(xbuf[:, qt], xbuf[:, qt], ops)
                if use_b2:
                    nc.vector.tensor_add(xbuf[:, qt], xbuf[:, qt], b2r)
                nc.sync.dma_start(out_d.ap()[:, qt], xbuf[:, qt])

        if repeat == 1:
            _emit_iteration()
        else:
            with tc.For_i(0, repeat, 1):
                _emit_iteration()

    # The act-table-load inserter picks the FIRST set containing each needed
    # function; with the default order Exp finds `exp_and_others` (no Ln) and
    # Ln finds `natural_log` (no Exp), thrashing ~45 loads (~2.7us each).
    # Putting `natural_log_exp_and_others` (Exp+Ln+Copy+Square+Relu) first
    # collapses this to one load. Patch is scoped to this compile.
    orig_gat = bacc.get_activation_tables
    pref = "natural_log_exp_and_others"

    def _gat_pref(arch):
        t = dict(orig_gat(arch))
        return {pref: t[pref], **t} if pref in t else t

    bacc.get_activation_tables = _gat_pref
    try:
        nc.compile()
    finally:
        bacc.get_activation_tables = orig_gat
    # The emitted act_func_set_id indexes the tables list the inserter saw
    # (our reordered one), but walrus/NRT resolve it against act_info.json's
    # original order — remap each load's id back to the original index.
    names_re = list(_gat_pref(nc.m.arch).keys())
    names_orig = list(orig_gat(nc.m.arch).keys())
    for blk in nc.main_func.blocks:
        for inst in blk.instructions:
            if type(inst).__name__ == "InstLoadActFuncSet":
                inst.act_func_set_id = names_orig.index(
                    names_re[inst.act_func_set_id]
                )
    return nc


def _tile_p(a, inner=P):
    """[N*P, ...] -> [P, N, ...] so each SBUF partition's data is contiguous."""
    return np.ascontiguousarray(
        a.reshape(a.shape[0] // inner, inner, *a.shape[1:]).swapaxes(0, 1)
    )


def _prep_core_inputs(x, encoder_output, mask, W1, b1, W2, b2, ln_aff, flags):
    """Build per-core in_maps (host-side sharding + layout prep)."""
    import ml_dtypes

    a1, a2, a3, use_b1, use_b2 = flags
    f8 = ml_dtypes.float8_e4m3fn
    in_maps = []
    for c in range(N_CORES):
        b, half = c // 2, c % 2
        q0 = half * Q
        perm = np.concatenate(
            [np.arange(q0, q0 + Q), np.arange((1 - half) * Q, (1 - half) * Q + Q)]
        )
        xb = np.ascontiguousarray(x[b][perm]).astype(np.float32)
        enc = encoder_output[b].astype(np.float32)
        encT = np.ascontiguousarray(enc.T)
        encv = np.zeros((S, H, 80), f8)
        encv[:, :, :DK] = enc.reshape(S, H, DK).astype(f8)
        encv[:, :, DK] = 1.0
        m = mask[b, 0][q0 : q0 + Q][:, perm]  # [Q, S] in permuted key order
        mbias = ((m.T - 1) * (-MASK_BIAS)).astype(ml_dtypes.bfloat16)
        mmult = np.ascontiguousarray(m.T).astype(ml_dtypes.bfloat16)
        w1 = W1.astype(ml_dtypes.bfloat16)
        w2 = W2.astype(ml_dtypes.bfloat16)
        im = {
            "x": xb,
            "xq": _tile_p(xb[0:Q]),
            "encT": _tile_p(encT.astype(ml_dtypes.bfloat16)),
            "encv": _tile_p(encv),
            "maskT": _tile_p(np.ascontiguousarray(mbias)),
            "maskM": _tile_p(mmult),
            "w1": _tile_p(w1),
            "w2": _tile_p(w2),
        }
        for i, a in ((1, a1), (2, a2), (3, a3)):
            if a:
                g, bta = ln_aff[i]
                im[f"ln{i}gr"] = np.tile(g.astype(np.float32)[None, :], (P, 1))
                im[f"ln{i}br"] = np.tile(bta.astype(np.float32)[None, :], (P, 1))
        if use_b1:
            im["b1t"] = np.ascontiguousarray(
                b1.astype(np.float32).reshape(DFF // P, P).T
            )
        if use_b2:
            im["b2r"] = np.tile(b2.astype(np.float32)[None, :], (P, 1))
        in_maps.append(im)
    return in_maps


def kernel(x, encoder_output, mask, ln1_g, ln1_b, ln2_g, ln2_b, ln3_g, ln3_b,
           W1, b1, W2, b2):
    from concourse import bass_utils

    x = np.asarray(x)
    encoder_output = np.asarray(encoder_output)
    mask = np.asarray(mask)
    ln = {
        1: (np.asarray(ln1_g), np.asarray(ln1_b)),
        2: (np.asarray(ln2_g), np.asarray(ln2_b)),
        3: (np.asarray(ln3_g), np.asarray(ln3_b)),
    }
    flags = (
        *(not (np.all(ln[i][0] == 1.0) and np.all(ln[i][1] == 0.0)) for i in (1, 2, 3)),
        bool(np.any(np.asarray(b1) != 0.0)),
        bool(np.any(np.asarray(b2) != 0.0)),
    )
    nc = _build_graph(*flags)
    in_maps = _prep_core_inputs(
        x, encoder_output, mask, np.asarray(W1), np.asarray(b1), np.asarray(W2),
        np.asarray(b2), ln, flags,
    )
    res = bass_utils.run_bass_kernel_spmd(nc, in_maps, core_ids=list(range(N_CORES)))
    out = np.empty((B, S, D), np.float32)
    for c in range(N_CORES):
        b, half = c // 2, c % 2
        # out dram layout is [P, TQ, D] -> token-major [Q, D]
        o = res.results[c]["out"].swapaxes(0, 1).reshape(Q, D)
        out[b, half * Q : (half + 1) * Q] = o
    return out
